# revision 69
# baseline (speedup 1.0000x reference)
"""Trainium2 Bass kernel for BPT attention wrapper with alibi (head-axis attention).

Sharding: 8 cores = 2 batches x 4 sequence-quarters (512 positions each).
Cross-core: one AllReduce of per-head Gram matrices G_h = Q_h^T Q_h within each
4-core batch group (pinv(Q) = G^{-1} Q^T needs full-sequence G); the AllReduce
is kicked off after the q third of the QKV GEMM and hides behind the k/v thirds.

Math per (b,s) position, per head pair (i,j):
  scores[i,j] = (q_i . k_j + sqrt(D) * alibi[j,s] * (q_i . z_j)) / D
  z_j = G_j^{-1} q_j     (Newton-Schulz inverse on device)
  attn = softmax_j(scores);  ctx_i = sum_j attn[i,j] v_j
  out = ctx @ dense_w.T + dense_b + residual
"""
import numpy as np
import ml_dtypes

import concourse.bass as bass
import concourse.mybir as mybir
from concourse import bacc, bass_isa
from concourse.tile import TileContext
from concourse.bass_utils import run_bass_kernel_spmd
from concourse.masks import make_identity

F32 = mybir.dt.float32
F32R = mybir.dt.float32r
BF16 = mybir.dt.bfloat16
AL = mybir.AluOpType
ACTF = mybir.ActivationFunctionType

B, S, H, D = 2, 2048, 16, 128
HID = H * D
N_CORES = 8
POS = 512                 # positions per core
NEWTON_ITERS = 5
MASK_BIG = 3000.0         # exp((s - MASK_BIG)/D) == 0 for any realistic score


def chunk_order():
    """QKV output chunks (48 x 128 rows) reordered so all q chunks come first
    (then k, then v) — lets the G AllReduce start after 1/3 of the GEMM."""
    return [m for t in range(3) for m in range(t, 48, 3)]


def build_bass(pos=POS, use_cc=True, newton_iters=NEWTON_ITERS, groups=None,
               reps=1, wdt=BF16):
    """Build the per-core Bass program. pos must be a multiple of 128."""
    nblk = pos // 8               # 8-position attention blocks
    ngrp = nblk // 4              # phase-B processes 4 blocks per group
    nchunk = HID // 128           # 16 contraction chunks
    n_rw = 3 * HID // 512         # 12 row windows of 512 qkv rows
    order = chunk_order()

    nc = bacc.Bacc()

    hsT3 = nc.dram_tensor("hsT3", (128, nchunk * pos), wdt, kind="ExternalInput")
    qkvw = nc.dram_tensor("qkvw", (128, n_rw * nchunk * 512), wdt, kind="ExternalInput")
    densew = nc.dram_tensor("densew", (128, nchunk * 4 * 512), wdt, kind="ExternalInput")
    resT = nc.dram_tensor("resT", (128, 16 * pos), BF16, kind="ExternalInput")
    albc = nc.dram_tensor("albc", (128, H * pos), BF16, kind="ExternalInput")
    maskb = nc.dram_tensor("maskb", (128, 128), BF16, kind="ExternalInput")
    qkvb48 = nc.dram_tensor("qkvb48", (128, 48), F32, kind="ExternalInput")
    denseb16 = nc.dram_tensor("denseb16", (128, 16), F32, kind="ExternalInput")
    outT = nc.dram_tensor("outT", (16, 128, pos), F32, kind="ExternalOutput")

    with TileContext(nc) as tc:
      for _rep in range(reps):
            with (
                tc.tile_pool(name="per", bufs=1) as per,
                tc.tile_pool(name="dram", bufs=1, space="DRAM") as dram,
            ):
                # persistent SBUF tensors. Matmul operands need single-free-dim
                # APs on HW, so attention inputs live in block-major layout
                # [d, (blk, j, p)]; q additionally head-major for G/z.
                t_qtp = per.tile([128, H * pos], BF16, tag="qtp")
                t_qti = per.tile([128, H * pos], BF16, tag="qti")
                t_kti = per.tile([128, H * pos], BF16, tag="kti")
                t_vti = per.tile([128, H * pos], BF16, tag="vti")
                t_zti = per.tile([128, H * pos], BF16, tag="zti")
                t_maskb = per.tile([128, 128], BF16, tag="maskb")
                t_dnb = per.tile([128, 16], F32, tag="dnb")
                t_id16 = per.tile([128, 128], BF16, tag="id16")
                t_id4 = per.tile([128, 512], BF16, tag="id4")
                t_ones = per.tile([128, 128], BF16, tag="ones")
                t_vb = per.tile([128, H * pos], BF16, tag="vb")

                nc.scalar.dma_start(t_maskb[:], maskb[:])
                nc.scalar.dma_start(t_dnb[:], denseb16[:])
                make_identity(nc, t_id16[:])
                for rep4 in range(4):
                    make_identity(nc, t_id4[:, rep4 * 128:(rep4 + 1) * 128])
                nc.gpsimd.memset(t_ones[:], 1.0)

                def bmview(t, h):
                    # [d, k, p] strided write view of block-major t at head h
                    return t[:].rearrange("d (k j p) -> d k j p",
                                          j=H, p=8)[:, :, h, :]

                # ---- pool spanning phases A + A2 (pinv working set) ----
                spa_cm = tc.tile_pool(name="spa", bufs=1)
                spa = spa_cm.__enter__()
                t_albc = spa.tile([128, H * pos], BF16, tag="albc")
                t_g32 = spa.tile([128, H * 128], F32, tag="g32")
                t_g16 = spa.tile([128, H * 128], BF16, tag="g16")
                t_w = spa.tile([128, H * 128], BF16, tag="wall")
                t_x = [spa.tile([128, H * 128], BF16, tag=f"x{i}", name=f"t_x{i}")
                       for i in range(2)]
                t_diag = spa.tile([128, H], F32, tag="diag")
                t_cbc = spa.tile([128, H], F32, tag="cbc")

                # ---------------- Phase A: QKV projection (+ per-head G) ----------------
                with (
                    tc.tile_pool(name="a_hs", bufs=1) as a_hs,
                    tc.tile_pool(name="a_w", bufs=2) as a_w,
                    tc.tile_pool(name="a_ps", bufs=1, space="PSUM") as a_ps,
                    tc.tile_pool(name="g_ps", bufs=2, space="PSUM") as g_ps,
                    tc.tile_pool(name="g_sb", bufs=3) as g_sb,
                ):
                    # hidden states in two tiles so the first matmuls start early
                    # (one DMA per tile: a shared tensor would make the two DMA
                    # completions indistinguishable to readers)
                    t_hsA = a_hs.tile([128, 2 * pos], wdt, tag="hsA")
                    t_hsB = a_hs.tile([128, 14 * pos], wdt, tag="hsB")
                    nc.sync.dma_start(t_hsA[:], hsT3[:, 0:2 * pos])
                    nc.sync.dma_start(t_hsB[:], hsT3[:, 2 * pos:16 * pos])

                    def hs_chunk(cc):
                        if cc < 2:
                            return t_hsA[:, cc * pos:(cc + 1) * pos]
                        return t_hsB[:, (cc - 2) * pos:(cc - 1) * pos]

                    t_qkvb = a_hs.tile([128, 48], F32, tag="qkvb")
                    nc.sync.dma_start(t_qkvb[:], qkvb48[:])

                    def emit_g_head(h):
                        # qp4 = q_h^T in 4 chunks (one PSUM bank), 1 copy, 4 G-accum mms
                        qp4 = g_ps.tile([128, 512], F32, tag="qp4")
                        for ccc in range(4):
                            nc.tensor.matmul(
                                qp4[:, ccc * 128:(ccc + 1) * 128],
                                t_qtp[:, h * pos + ccc * 128: h * pos + (ccc + 1) * 128],
                                t_id16[:], start=True, stop=True)
                        qch = g_sb.tile([128, 512], BF16, tag="qch")
                        nc.scalar.activation(qch[:], qp4[:], ACTF.Copy)
                        gp = g_ps.tile([128, 128], F32, tag="gps")
                        for ccc in range(4):
                            nc.tensor.matmul(gp[:], qch[:, ccc * 128:(ccc + 1) * 128],
                                             qch[:, ccc * 128:(ccc + 1) * 128],
                                             start=(ccc == 0), stop=(ccc == 3))
                        nc.vector.tensor_copy(t_g32[:, h * 128:(h + 1) * 128], gp[:])

                    t_aw0a = a_hs.tile([128, 2 * 512], wdt, tag="aw0a")
                    nc.sync.dma_start(t_aw0a[:], qkvw[:, 0:1024])
                    wincols = nchunk * 512
                    for rw in range(n_rw):
                        psums = [a_ps.tile([128, pos], F32, tag=f"aps{rt}", name=f"aps{rw}_{rt}") for rt in range(4)]
                        t_aw = a_w.tile([128, nchunk * 512], wdt, tag="aw")
                        if rw == 0:
                            nc.sync.dma_start(t_aw[:, 1024:], qkvw[:, 1024:wincols])
                        else:
                            nc.sync.dma_start(t_aw[:], qkvw[:, rw * wincols:(rw + 1) * wincols])

                        def w_chunk(cc, rt, _rw=rw, _aw=t_aw):
                            if _rw == 0 and cc < 2:
                                return t_aw0a[:, cc * 512 + rt * 128: cc * 512 + (rt + 1) * 128]
                            return _aw[:, cc * 512 + rt * 128: cc * 512 + (rt + 1) * 128]

                        for rt in range(4):
                            for cc in range(nchunk):
                                nc.tensor.matmul(
                                    psums[rt][:],
                                    w_chunk(cc, rt),
                                    hs_chunk(cc),
                                    start=(cc == 0),
                                    stop=(cc == nchunk - 1))
                        for rt in range(4):
                            m = order[rw * 4 + rt]
                            h, t = divmod(m, 3)
                            psv = psums[rt][:].rearrange("d (k p) -> d k p", p=8)
                            bias = t_qkvb[:, m:m + 1]
                            if t == 0:
                                nc.scalar.activation(t_qtp[:, h * pos:(h + 1) * pos],
                                                     psums[rt][:], ACTF.Identity,
                                                     bias=bias)
                                nc.vector.tensor_scalar_add(bmview(t_qti, h), psv,
                                                            bias)
                                emit_g_head(h)
                            else:
                                dst = t_kti if t == 1 else t_vti
                                nc.scalar.activation(bmview(dst, h), psv,
                                                     ACTF.Identity, bias=bias)
                        if use_cc and rw == 3:
                            # all 16 G heads done: AllReduce overlaps the k/v GEMM.
                            # both DMAs ride the Pool queue (collective lives there)
                            # so they never head-of-line-block SP weight streaming.
                            ccin = dram.tile([128, H * 128], F32, tag="ccin")
                            ccout = dram.tile([128, H * 128], F32, tag="ccout")
                            nc.gpsimd.dma_start(ccin[:], t_g32[:])
                            nc.gpsimd.collective_compute(
                                "AllReduce", AL.add,
                                replica_groups=groups or [[0, 1, 2, 3], [4, 5, 6, 7]],
                                ins=[ccin[:]], outs=[ccout[:]])
                            nc.gpsimd.dma_start(t_g32[:], ccout[:])

                # v^T for all 64 blocks: one tiled DMA transpose (XBAR),
                # runs as soon as the last v chunk lands
                nc.scalar.dma_start_transpose(
                    t_vb[:].rearrange("p (b c) -> p b c", c=128), t_vti[:])

                # ---------------- Phase A2: Newton + z ----------------
                # everything from here to end-of-z waits on the AllReduce;
                # deprioritize it so the scheduler never slots it ahead of
                # independent phase-A work in any engine stream (head-of-line)
                a2_prio = tc.tile_wait_until(1.0)
                a2_prio.__enter__()
                nc.sync.dma_start(t_albc[:], albc[:])   # needed by z only
                nc.vector.tensor_copy(t_g16[:], t_g32[:])

                with (
                    tc.tile_pool(name="n_sb", bufs=4) as n_sb,
                    tc.tile_pool(name="n_ps", bufs=2, space="PSUM") as n_ps,
                ):
                    # safe init: c_h = 1 / ||G_h||_inf  (max abs row sum >= lambda_max)
                    for h in range(H):
                        nc.vector.tensor_reduce(
                            t_diag[:, h:h + 1], t_g32[:, h * 128:(h + 1) * 128],
                            axis=mybir.AxisListType.X, op=AL.add,
                            apply_absolute_value=True)
                    nc.gpsimd.partition_all_reduce(t_cbc[:], t_diag[:], channels=128,
                                                   reduce_op=bass_isa.ReduceOp.max)
                    nc.vector.reciprocal(t_cbc[:], t_cbc[:])

                    for h in range(H):
                        nc.vector.tensor_scalar_mul(
                            t_x[0][:, h * 128:(h + 1) * 128], t_id16[:],
                            t_cbc[:, h:h + 1])
                    for it in range(newton_iters):
                        xc = t_x[it % 2]
                        xn = t_w if it == newton_iters - 1 else t_x[1 - it % 2]
                        for g4 in range(4):
                            sl4 = slice(g4 * 512, (g4 + 1) * 512)
                            yp4 = n_ps.tile([128, 512], F32, tag="yps")
                            for hh in range(4):
                                h = g4 * 4 + hh
                                nc.tensor.matmul(
                                    yp4[:, hh * 128:(hh + 1) * 128],
                                    t_g16[:, h * 128:(h + 1) * 128],
                                    xc[:, h * 128:(h + 1) * 128],
                                    start=True, stop=True)
                            ysb4 = n_sb.tile([128, 512], BF16, tag="ysb")
                            nc.scalar.activation(ysb4[:], yp4[:], ACTF.Copy)
                            zp4 = n_ps.tile([128, 512], F32, tag="zps")
                            for hh in range(4):
                                h = g4 * 4 + hh
                                nc.tensor.matmul(
                                    zp4[:, hh * 128:(hh + 1) * 128],
                                    xc[:, h * 128:(h + 1) * 128],
                                    ysb4[:, hh * 128:(hh + 1) * 128],
                                    start=True, stop=True)
                            nc.vector.scalar_tensor_tensor(
                                xn[:, sl4], xc[:, sl4], 2.0, zp4[:],
                                op0=AL.mult, op1=AL.subtract)

                    # z_h = W_h @ q_h, prescaled by sqrt(D)*alibi into zti
                    for h in range(H):
                        zp = n_ps.tile([128, pos], F32, tag="ztps")
                        nc.tensor.matmul(zp[:], t_w[:, h * 128:(h + 1) * 128],
                                         t_qtp[:, h * pos:(h + 1) * pos],
                                         start=True, stop=True)
                        nc.vector.tensor_tensor(
                            bmview(t_zti, h),
                            zp[:].rearrange("d (k p) -> d k p", p=8),
                            bmview(t_albc, h), op=AL.mult)

                a2_prio.__exit__(None, None, None)
                spa_cm.__exit__(None, None, None)

                # ---------------- Phase B/C span: ctxT + dense-weight prefetch ----------------
                spb_cm = tc.tile_pool(name="spb", bufs=1)
                spb = spb_cm.__enter__()
                t_ctxT = spb.tile([128, H * pos], wdt, tag="ctxT")
                t_cws = [spb.tile([128, 16 * 512], wdt, tag=f"cw{i}", name=f"t_cw{i}")
                         for i in range(4)]
                for i in range(4):
                    nc.sync.dma_start(t_cws[i][:],
                                      densew[:, i * 16 * 512:(i + 1) * 16 * 512])
                t_res = spb.tile([128, 16 * pos], BF16, tag="res")
                nc.sync.dma_start(t_res[:], resT[:])

                # ---------------- Phase B: block attention, 4 blocks per group ----------------
                with (
                    tc.tile_pool(name="b_sb", bufs=4) as b_sb,
                    tc.tile_pool(name="b_ps", bufs=3, space="PSUM") as b_ps,
                    tc.tile_pool(name="b_psd", bufs=2, space="PSUM") as b_psd,
                ):
                    ctx_v = t_ctxT[:].rearrange("d (i s) -> d i s", i=H)
                    for bg in range(ngrp):
                        blks = [bg * 4 + b for b in range(4)]
                        # scores (+ masked-out cross-position terms via -MASK_BIG)
                        sp4 = b_ps.tile([128, 512], F32, tag="sps")
                        nc.tensor.matmul(sp4[:], t_maskb[:], t_id4[:],
                                         start=True, stop=False)
                        for b, blk in enumerate(blks):
                            bsl = slice(b * 128, (b + 1) * 128)
                            ksl = slice(blk * 128, (blk + 1) * 128)
                            nc.tensor.matmul(sp4[:, bsl], t_kti[:, ksl],
                                             t_qti[:, ksl],
                                             start=False, stop=False,
                                             skip_group_check=True)
                            nc.tensor.matmul(sp4[:, bsl], t_zti[:, ksl],
                                             t_qti[:, ksl],
                                             start=False, stop=(b == 3),
                                             skip_group_check=True)
                        esb4 = b_sb.tile([128, 512], BF16, tag="esb")
                        nc.scalar.activation(esb4[:], sp4[:], ACTF.Exp,
                                             scale=1.0 / float(D))

                        # denominators, broadcast over partitions by all-ones stationary
                        dn = b_psd.tile([128, 512], F32, tag="dps")
                        nc.tensor.matmul(dn[:], t_ones[:], esb4[:],
                                         start=True, stop=True)
                        rec4 = b_sb.tile([128, 512], BF16, tag="rec")
                        with nc.allow_low_precision(reason="softmax weights to bf16"):
                            nc.vector.reciprocal(rec4[:], dn[:])
                        esn4 = b_sb.tile([128, 512], BF16, tag="esn")
                        nc.vector.tensor_tensor(esn4[:], esb4[:], rec4[:],
                                                op=AL.mult)

                        # ctx^T directly: [d, (i,p)] per block
                        cp4 = b_ps.tile([128, 512], F32, tag="cps")
                        for b, blk in enumerate(blks):
                            bsl = slice(b * 128, (b + 1) * 128)
                            nc.tensor.matmul(cp4[:, bsl],
                                             t_vb[:, blk * 128:(blk + 1) * 128],
                                             esn4[:, bsl], start=True, stop=True)
                        nc.scalar.activation(
                            ctx_v[:, :, bg * 32:(bg + 1) * 32]
                            .rearrange("d i (b p) -> d b i p", b=4),
                            cp4[:].rearrange("d (b i p) -> d b i p", b=4, i=H),
                            ACTF.Copy)

                # ---------------- Phase C: dense + residual ----------------
                with (
                    tc.tile_pool(name="c_ps", bufs=1, space="PSUM") as c_ps,
                    tc.tile_pool(name="c_sb", bufs=3) as c_sb,
                ):
                    for ow in range(4):
                        psums = [c_ps.tile([128, pos], F32, tag=f"cps{oc}", name=f"cps{ow}_{oc}") for oc in range(4)]
                        for oc in range(4):
                            for cc in range(16):
                                n = cc * 4 + ow
                                base = (n % 16) * 512 + oc * 128
                                nc.tensor.matmul(
                                    psums[oc][:],
                                    t_cws[n // 16][:, base: base + 128],
                                    t_ctxT[:, cc * pos:(cc + 1) * pos],
                                    start=(cc == 0),
                                    stop=(cc == 15))
                        for oc in range(4):
                            ot = ow * 4 + oc
                            osb = c_sb.tile([128, pos], F32, tag="osb")
                            nc.vector.scalar_tensor_tensor(
                                osb[:], psums[oc][:], t_dnb[:, ot:ot + 1],
                                t_res[:, ot * pos:(ot + 1) * pos],
                                op0=AL.add, op1=AL.add)
                            nc.sync.dma_start(outT[ot], osb[:])
                spb_cm.__exit__(None, None, None)
    nc.compile()
    return nc


_CACHED = {}


def _get_nc(pos=POS, use_cc=True):
    key = (pos, use_cc)
    if key not in _CACHED:
        _CACHED[key] = build_bass(pos=pos, use_cc=use_cc)
    return _CACHED[key]


def make_in_maps(hidden_states, residual, alibi, qkv_w, qkv_b, dense_w, dense_b,
                 pos=POS, n_cores=N_CORES, cores_per_batch=4,
                 wdt_np=ml_dtypes.bfloat16):
    nchunk = HID // 128
    n_rw = 3 * HID // 512
    order = chunk_order()

    qkv_wT = np.ascontiguousarray(qkv_w.T).astype(np.float32)     # [HID, 3HID]
    # per-chunk gather into the q-first order, then partition-major flat
    # [d, (rw, cc, rt, col)] so each row-window loads with one contiguous DMA
    qkvw_c = qkv_wT.reshape(nchunk, 128, 48, 128)[:, :, order, :]
    qkvw_t = np.ascontiguousarray(
        qkvw_c.reshape(nchunk, 128, n_rw, 4, 128).transpose(1, 2, 0, 3, 4)
    ).reshape(128, n_rw * nchunk * 512).astype(wdt_np)
    dense_wT = np.ascontiguousarray(dense_w.T).astype(np.float32)  # [HID, HID]
    densew_t = np.ascontiguousarray(
        dense_wT.reshape(nchunk, 128, 4, 512).transpose(1, 0, 2, 3)
    ).reshape(128, nchunk * 4 * 512).astype(wdt_np)
    qkvb = np.ascontiguousarray(qkv_b.reshape(48, 128).T).astype(np.float32)
    dnb = np.ascontiguousarray(dense_b.reshape(16, 128).T).astype(np.float32)
    # additive pre-exp mask: 0 on same-position (p==p') pairs, -MASK_BIG elsewhere
    pp = np.arange(8)
    mask = (pp[None, :, None, None] == pp[None, None, None, :])
    mask = np.broadcast_to(mask, (16, 8, 16, 8)).reshape(128, 128)
    maskb_host = np.ascontiguousarray(
        ((mask.astype(np.float32) - 1.0) * MASK_BIG).astype(ml_dtypes.bfloat16))

    in_maps = []
    for c in range(n_cores):
        b = c // cores_per_batch
        sq = c % cores_per_batch
        ssl = slice(sq * pos, (sq + 1) * pos)
        hsT = np.ascontiguousarray(hidden_states[b, ssl, :].T).astype(np.float32)
        hsT3 = np.ascontiguousarray(hsT.reshape(nchunk, 128, pos).transpose(1, 0, 2)
                                    .reshape(128, nchunk * pos)).astype(wdt_np)
        rT = np.ascontiguousarray(residual[b, ssl, :].T).astype(np.float32)
        rT3 = np.ascontiguousarray(
            rT.reshape(16, 128, pos).transpose(1, 0, 2).reshape(128, 16 * pos)
        ).astype(ml_dtypes.bfloat16)
        # albc[d, (k, j, p)] = sqrt(D) * alibi[b*H + j, 0, sq*pos + k*8 + p]
        al = np.asarray(alibi)[b * H:(b + 1) * H, 0, ssl]          # [H, pos]
        al_bjp = (np.sqrt(float(D)) * al).reshape(H, pos // 8, 8).transpose(1, 0, 2)
        albc_host = np.ascontiguousarray(np.broadcast_to(
            al_bjp.reshape(1, H * pos), (128, H * pos))
        ).astype(ml_dtypes.bfloat16)
        in_maps.append({
            "hsT3": hsT3,
            "qkvw": qkvw_t,
            "densew": densew_t,
            "resT": rT3,
            "albc": albc_host,
            "maskb": maskb_host,
            "qkvb48": qkvb,
            "denseb16": dnb,
        })
    return in_maps


def kernel(hidden_states, residual, alibi, attention_mask, qkv_w, qkv_b,
           dense_w, dense_b):
    hidden_states = np.asarray(hidden_states, dtype=np.float32)
    residual = np.asarray(residual, dtype=np.float32)
    alibi = np.asarray(alibi, dtype=np.float32)
    qkv_w = np.asarray(qkv_w, dtype=np.float32)
    qkv_b = np.asarray(qkv_b, dtype=np.float32)
    dense_w = np.asarray(dense_w, dtype=np.float32)
    dense_b = np.asarray(dense_b, dtype=np.float32)

    nc = _get_nc()
    in_maps = make_in_maps(hidden_states, residual, alibi, qkv_w, qkv_b,
                           dense_w, dense_b)
    res = run_bass_kernel_spmd(nc, in_maps, core_ids=list(range(N_CORES)))
    out = np.empty((B, S, HID), np.float32)
    for c in range(N_CORES):
        b, sq = c // 4, c % 4
        oT = res.results[c]["outT"]          # [16, 128, POS]
        out[b, sq * POS:(sq + 1) * POS, :] = oT.reshape(HID, POS).T
    return out
